# revision 14
# baseline (speedup 1.0000x reference)
# MoE top-2 routing kernel for 8 Trainium2 NeuronCores (expert-parallel).
#
# Problem (hardcoded shapes): T=2048 tokens, D=2048 model dim, F=4096 ffn dim,
# E=8 experts, top-2 routing with renormalized softmax weights.
#
# Sharding: one expert per core. The host does the router (fp32 logits ->
# top-2 selection + renormalized softmax weights; selection is numerically
# unambiguous: min 2nd-vs-3rd prob gap ~9e-5, orders of magnitude above fp32
# matmul noise), gathers each expert's tokens into a transposed buffer
# xb [D, C] (C = max expert load rounded up to 8), and zero-pads the tail.
# Zero columns are harmless: MLP(0) = 0 and the host ignores pad columns.
#
# Device structure (all matmuls weights-STATIONARY, tokens moving): PE cost
# scales with the actual token count C (536 here) instead of 128-padded
# token tiles (5*128=640 in the v1 layout), and gate/up naturally produce
# [f, t] layout so the down matmul needs no PE transposes at all.
#   gate/up: pg[f128, C] = sum_d wg[d,f128].T @ x[d, C]   (per 128-f tile)
#   h[f, t] = silu(g) * u  (scalar+vector, bf16)
#   down:    y[d128, C] += sum_f wd[f,d128].T @ h[f, C]   (PSUM per 8-f group,
#            accumulated into SBUF ya, router-weight scaled, DMA'd out [D, C])
# Moving-dim chunks of ~C/2 <= 512 fp32 PSUM bank columns. bf16 matmuls
# stream ~1 col/cycle at the 2.0 GHz PE clock; LDWEIGHTS pipelines under the
# matmuls via the PE reorder window (trace: 139 ns per 272-col matmul, zero
# scheduling stalls).
#
# DMA: each DMA instruction costs ~700ns of issue on its queue engine and
# carries ~128 descriptors in parallel across 16 HW engines, so small tiles
# cap aggregate bandwidth (~175 GB/s at 128KB/instr). Weights load as ONE
# instruction per 512-f-chunk ([128, 16*512] rearranged), x as 4, and the
# wu/wd/wvb loads are emitted after the first gate matmuls so the startup
# critical path is just x + the first gate weight chunk.

import os
import numpy as np
import ml_dtypes

_BF16NP = ml_dtypes.bfloat16

import concourse.bass as bass
import concourse.bacc as bacc
import concourse.mybir as mybir
import concourse.tile as tile
from concourse import bass_utils

FP32 = mybir.dt.float32
BF16 = mybir.dt.bfloat16
AX = mybir.AxisListType
ALU = mybir.AluOpType
ACTF = mybir.ActivationFunctionType

T, D, F, E = 2048, 2048, 4096, 8
NCORES = 8
ND = D // 128    # 16 d-tiles (contraction for gate/up; output tiles for down)
NF = F // 128    # 32 f-tiles
G = 8            # f-tiles per down-accumulation group
NG = NF // G     # 4 groups


def _chunks8(C):
    """Split C token columns into PSUM-bank chunks (<=512 fp32, mult of 8)."""
    nch = (C + 511) // 512
    out, rem, c0 = [], C, 0
    for i in range(nch):
        cn = -(-(rem // (nch - i)) // 8) * 8
        cn = min(cn, rem)
        out.append((c0, cn))
        c0 += cn
        rem -= cn
    return out


def build_program(C):
    chunks = _chunks8(C)
    nc = bacc.Bacc(
        "TRN2",
        target_bir_lowering=False,
        debug=False,
        enable_asserts=False,
        num_devices=NCORES,
    )
    xb_d = nc.dram_tensor("xb", [D, C], BF16, kind="ExternalInput").ap()
    wvb_d = nc.dram_tensor("wvb", [128, C], FP32, kind="ExternalInput").ap()
    wg_d = nc.dram_tensor("wg", [D, F], BF16, kind="ExternalInput").ap()
    wu_d = nc.dram_tensor("wu", [D, F], BF16, kind="ExternalInput").ap()
    wd_d = nc.dram_tensor("wd", [F, D], BF16, kind="ExternalInput").ap()
    y_d = nc.dram_tensor("y", [D, C], FP32, kind="ExternalOutput").ap()

    with tile.TileContext(nc) as tc:
        with (
            tc.tile_pool(name="const", bufs=1) as const_pool,
            tc.tile_pool(name="x", bufs=1) as x_pool,
            tc.tile_pool(name="ya", bufs=1) as ya_pool,
            tc.tile_pool(name="h", bufs=2) as h_pool,
            tc.tile_pool(name="wgu", bufs=4) as wgu_pool,
            tc.tile_pool(name="wdp", bufs=2) as wd_pool,
            tc.tile_pool(name="tmp", bufs=2) as tmp_pool,
            tc.tile_pool(name="ps", bufs=6, space="PSUM") as ps_pool,
            tc.tile_pool(name="psy", bufs=2, space="PSUM") as psy_pool,
        ):
            # ---- PE warm-up: dummy matmuls on memset data while the first
            # DMAs stream in, so the HAM clock-gate reaches 8/8 (full rate)
            # before the real matmuls start. PE is otherwise idle here. ----
            warm = const_pool.tile([128, 640], BF16, tag="warm", name="warm")
            nc.vector.memset(warm[:], 0.0)
            for _ in range(20):
                pw = ps_pool.tile([128, 512], FP32, tag="ps", name="ps")
                nc.tensor.matmul(pw[:], warm[:, :128], warm[:, 128:640],
                                 start=True, stop=True)

            # ---- startup DMAs: weights on the Sync queue, x on the GpSimd
            # queue — two independent in-order queues (~350 GB/s each) so the
            # gate weights and x stream in parallel. fc0's wg/wu are split in
            # halves so the first matmuls can start after ~1MB. ----
            wg_sb = {}   # fc -> [128, 16*512] tile (all 16 d-tiles merged)
            wu_sb = {}
            wd_sb = {}   # ft -> [128, 2048] tile

            def load_w0(dram, name):
                wt = wgu_pool.tile([128, ND * 512], BF16, tag="w", name=name)
                for half in range(2):
                    nc.sync.dma_start(
                        wt[:, half * 8 * 512:(half + 1) * 8 * 512]
                        .rearrange("p (n f) -> p n f", n=8),
                        dram[half * 1024:(half + 1) * 1024, :512]
                        .rearrange("(n p) f -> p n f", p=128),
                    )
                return wt

            wg_sb[0] = load_w0(wg_d, "wgt")
            wu_sb[0] = load_w0(wu_d, "wut")

            xt4 = [x_pool.tile([128, 4 * C], BF16, tag=f"xt{q}", name=f"xt{q}")
                   for q in range(4)]
            for q in range(4):
                nc.gpsimd.dma_start(
                    xt4[q][:].rearrange("p (n c) -> p n c", n=4),
                    xb_d[q * 512:(q + 1) * 512, :]
                    .rearrange("(n p) c -> p n c", p=128),
                )

            def xsl(d, c0, cn):
                q, r = divmod(d, 4)
                return xt4[q][:, r * C + c0:r * C + c0 + cn]

            wvb = const_pool.tile([128, C], FP32, tag="wvb", name="wvb")
            ya = [ya_pool.tile([128, C], FP32, tag=f"ya{dt}", name=f"ya{dt}")
                  for dt in range(ND)]

            def load_w(dram, fc, name):
                wt = wgu_pool.tile([128, ND * 512], BF16, tag="w", name=name)
                nc.sync.dma_start(
                    wt[:].rearrange("p (n f) -> p n f", n=ND),
                    dram[:, fc * 512:(fc + 1) * 512]
                    .rearrange("(n p) f -> p n f", p=128),
                )
                return wt

            def emit_down(gprev, hprev, j):
                """Down-matmul (d-tiles 2j, 2j+1) for f-group gprev."""
                f0 = gprev * G
                for dt in (2 * j, 2 * j + 1):
                    for (c0, cn) in chunks:
                        py = psy_pool.tile([128, max(cn for _, cn in chunks)],
                                           FP32, tag="py", name="py")
                        for k in range(G):
                            nc.tensor.matmul(
                                py[:, :cn],
                                wd_sb[f0 + k][:, dt * 128:(dt + 1) * 128],
                                hprev[k][:, c0:c0 + cn],
                                start=(k == 0), stop=(k == G - 1),
                            )
                        yslc = ya[dt][:, c0:c0 + cn]
                        if gprev == 0:
                            nc.scalar.copy(yslc, py[:, :cn])
                        else:
                            nc.vector.tensor_tensor(yslc, yslc, py[:, :cn],
                                                    op=ALU.add)
                    if gprev == NG - 1:
                        nc.vector.tensor_mul(ya[dt][:], ya[dt][:], wvb[:])
                        nc.gpsimd.dma_start(
                            y_d[dt * 128:(dt + 1) * 128, :], ya[dt][:])

            def emit_gate(fc, fo):
                pg = [ps_pool.tile([128, cn], FP32, tag="ps", name="ps")
                      for (c0, cn) in chunks]
                for d in range(ND):
                    wsl = wg_sb[fc][:, d * 512 + fo * 128:
                                    d * 512 + (fo + 1) * 128]
                    for ci, (c0, cn) in enumerate(chunks):
                        nc.tensor.matmul(
                            pg[ci][:], wsl, xsl(d, c0, cn),
                            start=(d == 0), stop=(d == ND - 1),
                        )
                st = tmp_pool.tile([128, C], FP32, tag="st", name="st",
                                   bufs=4)
                for ci, (c0, cn) in enumerate(chunks):
                    nc.scalar.activation(st[:, c0:c0 + cn], pg[ci][:],
                                         ACTF.Silu)
                return st

            def emit_up(fc, fo, st, j):
                pu = [ps_pool.tile([128, cn], FP32, tag="ps", name="ps")
                      for (c0, cn) in chunks]
                for d in range(ND):
                    wsl = wu_sb[fc][:, d * 512 + fo * 128:
                                    d * 512 + (fo + 1) * 128]
                    for ci, (c0, cn) in enumerate(chunks):
                        nc.tensor.matmul(
                            pu[ci][:], wsl, xsl(d, c0, cn),
                            start=(d == 0), stop=(d == ND - 1),
                        )
                ht = h_pool.tile([128, C], BF16, tag=f"h{j}", name=f"h{j}")
                for ci, (c0, cn) in enumerate(chunks):
                    nc.vector.tensor_mul(ht[:, c0:c0 + cn],
                                         st[:, c0:c0 + cn], pu[ci][:])
                ft = (fc * 4 + fo)
                wdt = wd_pool.tile([128, D], BF16, tag=f"wd{j}", name="wdt")
                nc.sync.dma_start(wdt[:], wd_d[ft * 128:(ft + 1) * 128, :])
                wd_sb[ft] = wdt
                return ht

            hprev = None
            for g in range(NG):
                hcur = []
                for j in range(G):
                    ft = g * G + j
                    fc, fo = divmod(ft, 4)
                    if fc == 0:
                        # fc0: run all 4 gate f-tiles before the first up so
                        # the PE has work while the up-weights stream in.
                        if j == 0:
                            sts0 = [emit_gate(0, ff) for ff in range(4)]
                        ht = emit_up(0, fo, sts0[fo], j)
                    else:
                        if fo == 0:
                            wg_sb[fc] = load_w(wg_d, fc, "wgt")
                        st = emit_gate(fc, fo)
                        # up-weights DMA goes behind this f-tile's gate MMs
                        if fo == 0:
                            wu_sb[fc] = load_w(wu_d, fc, "wut")
                        ht = emit_up(fc, fo, st, j)
                    if g == NG - 1 and j == 0:
                        nc.gpsimd.dma_start(wvb[:], wvb_d[:])
                    hcur.append(ht)
                    if hprev is not None:
                        emit_down(g - 1, hprev, j)
                hprev = hcur
            for j in range(G):
                emit_down(NG - 1, hprev, j)

    nc.compile()
    return nc


_PROGRAM_CACHE = {}


def _get_program(C):
    if C not in _PROGRAM_CACHE:
        _PROGRAM_CACHE[C] = build_program(C)
    return _PROGRAM_CACHE[C]


def _route_host(x_TD, router_w):
    """Host router: top-2 ids + renormalized softmax weights per token."""
    logits = (x_TD @ router_w).astype(np.float64)  # [T, E]
    logits -= logits.max(axis=1, keepdims=True)
    probs = np.exp(logits)
    probs /= probs.sum(axis=1, keepdims=True)
    order = np.argsort(-probs, axis=1, kind="stable")
    top2 = order[:, :2]
    w12 = np.take_along_axis(probs, top2, axis=1)
    w12 /= w12.sum(axis=1, keepdims=True)
    return top2, w12.astype(np.float32)


def kernel_with_results(x_TD, router_w, w_gate, w_up, w_down):
    x_TD = np.ascontiguousarray(x_TD, np.float32)
    router_w = np.ascontiguousarray(router_w, np.float32)

    top2, w12 = _route_host(x_TD, router_w)
    idx_lists, wt_lists = [], []
    for e in range(E):
        sel = top2 == e  # [T, 2]
        any_sel = sel.any(axis=1)
        ix = np.where(any_sel)[0]
        idx_lists.append(ix)
        wt_lists.append(w12[any_sel][sel[ix]])
    max_cnt = max(len(ix) for ix in idx_lists)
    C = max(64, -(-max_cnt // 8) * 8)

    nc = _get_program(C)

    xT = np.ascontiguousarray(x_TD.T).astype(_BF16NP)  # [D, T] bf16
    in_maps = []
    for e in range(E):
        ix = idx_lists[e]
        xb = np.zeros((D, C), _BF16NP)
        xb[:, :len(ix)] = xT[:, ix]
        wvb = np.zeros((1, C), np.float32)
        wvb[0, :len(ix)] = wt_lists[e]
        in_maps.append({
            "xb": xb,
            "wvb": np.ascontiguousarray(np.broadcast_to(wvb, (128, C))),
            "wg": w_gate[e].astype(_BF16NP),
            "wu": w_up[e].astype(_BF16NP),
            "wd": w_down[e].astype(_BF16NP),
        })

    try:
        res = bass_utils.run_bass_kernel_spmd(
            nc, in_maps, core_ids=list(range(NCORES))
        )
    except ModuleNotFoundError:
        # Tracing requested via env but the axon NTFF hook module is absent
        # in this image — rerun without tracing.
        os.environ["BASS_NEVER_TRACE"] = "1"
        res = bass_utils.run_bass_kernel_spmd(
            nc, in_maps, core_ids=list(range(NCORES))
        )

    out = np.zeros((T, D), np.float32)
    for e in range(E):
        ix = idx_lists[e]
        y = res.results[e]["y"]  # [D, C]
        out[ix] += y[:, :len(ix)].T
    return out, res


def kernel(**inputs):
    out, _ = kernel_with_results(**inputs)
    return out


# revision 15
# speedup vs baseline: 1.0010x; 1.0010x over previous
# MoE top-2 routing kernel for 8 Trainium2 NeuronCores (expert-parallel).
#
# Problem (hardcoded shapes): T=2048 tokens, D=2048 model dim, F=4096 ffn dim,
# E=8 experts, top-2 routing with renormalized softmax weights.
#
# Sharding: one expert per core. The host does the router (fp32 logits ->
# top-2 selection + renormalized softmax weights; selection is numerically
# unambiguous: min 2nd-vs-3rd prob gap ~9e-5, orders of magnitude above fp32
# matmul noise), gathers each expert's tokens into a transposed buffer
# xb [D, C] (C = max expert load rounded up to 8), and zero-pads the tail.
# Zero columns are harmless: MLP(0) = 0 and the host ignores pad columns.
#
# Device structure (all matmuls weights-STATIONARY, tokens moving): PE cost
# scales with the actual token count C (536 here) instead of 128-padded
# token tiles (5*128=640 in the v1 layout), and gate/up naturally produce
# [f, t] layout so the down matmul needs no PE transposes at all.
#   gate/up: pg[f128, C] = sum_d wg[d,f128].T @ x[d, C]   (per 128-f tile)
#   h[f, t] = silu(g) * u  (scalar+vector, bf16)
#   down:    y[d128, C] += sum_f wd[f,d128].T @ h[f, C]   (PSUM per 8-f group,
#            accumulated into SBUF ya, router-weight scaled, DMA'd out [D, C])
# Moving-dim chunks of ~C/2 <= 512 fp32 PSUM bank columns. bf16 matmuls
# stream ~1 col/cycle at the 2.0 GHz PE clock; LDWEIGHTS pipelines under the
# matmuls via the PE reorder window (trace: 139 ns per 272-col matmul, zero
# scheduling stalls).
#
# DMA: each DMA instruction costs ~700ns of issue on its queue engine and
# carries ~128 descriptors in parallel across 16 HW engines, so small tiles
# cap aggregate bandwidth (~175 GB/s at 128KB/instr). Weights load as ONE
# instruction per 512-f-chunk ([128, 16*512] rearranged), x as 4, and the
# wu/wd/wvb loads are emitted after the first gate matmuls so the startup
# critical path is just x + the first gate weight chunk.

import os
import numpy as np
import ml_dtypes

_BF16NP = ml_dtypes.bfloat16

import concourse.bass as bass
import concourse.bacc as bacc
import concourse.mybir as mybir
import concourse.tile as tile
from concourse import bass_utils

FP32 = mybir.dt.float32
BF16 = mybir.dt.bfloat16
AX = mybir.AxisListType
ALU = mybir.AluOpType
ACTF = mybir.ActivationFunctionType

T, D, F, E = 2048, 2048, 4096, 8
NCORES = 8
ND = D // 128    # 16 d-tiles (contraction for gate/up; output tiles for down)
NF = F // 128    # 32 f-tiles
G = 8            # f-tiles per down-accumulation group
NG = NF // G     # 4 groups


def _chunks8(C):
    """Split C token columns into PSUM-bank chunks (<=512 fp32, mult of 8)."""
    nch = (C + 511) // 512
    out, rem, c0 = [], C, 0
    for i in range(nch):
        cn = -(-(rem // (nch - i)) // 8) * 8
        cn = min(cn, rem)
        out.append((c0, cn))
        c0 += cn
        rem -= cn
    return out


def build_program(C):
    chunks = _chunks8(C)
    nc = bacc.Bacc(
        "TRN2",
        target_bir_lowering=False,
        debug=False,
        enable_asserts=False,
        num_devices=NCORES,
    )
    xb_d = nc.dram_tensor("xb", [D, C], BF16, kind="ExternalInput").ap()
    wvb_d = nc.dram_tensor("wvb", [128, C], FP32, kind="ExternalInput").ap()
    wg_d = nc.dram_tensor("wg", [D, F], BF16, kind="ExternalInput").ap()
    wu_d = nc.dram_tensor("wu", [D, F], BF16, kind="ExternalInput").ap()
    wd_d = nc.dram_tensor("wd", [F, D], BF16, kind="ExternalInput").ap()
    y_d = nc.dram_tensor("y", [D, C], FP32, kind="ExternalOutput").ap()

    with tile.TileContext(nc) as tc:
        with (
            tc.tile_pool(name="const", bufs=1) as const_pool,
            tc.tile_pool(name="x", bufs=1) as x_pool,
            tc.tile_pool(name="ya", bufs=1) as ya_pool,
            tc.tile_pool(name="h", bufs=2) as h_pool,
            tc.tile_pool(name="wgu", bufs=4) as wgu_pool,
            tc.tile_pool(name="wdp", bufs=2) as wd_pool,
            tc.tile_pool(name="tmp", bufs=2) as tmp_pool,
            tc.tile_pool(name="ps", bufs=6, space="PSUM") as ps_pool,
            tc.tile_pool(name="psy", bufs=2, space="PSUM") as psy_pool,
        ):
            # ---- PE warm-up: dummy matmuls on memset data while the first
            # DMAs stream in, so the HAM clock-gate reaches 8/8 (full rate)
            # before the real matmuls start. PE is otherwise idle here. ----
            warm = const_pool.tile([128, 640], BF16, tag="warm", name="warm")
            nc.vector.memset(warm[:], 0.0)
            for _ in range(16):
                pw = ps_pool.tile([128, 512], FP32, tag="ps", name="ps")
                nc.tensor.matmul(pw[:], warm[:, :128], warm[:, 128:640],
                                 start=True, stop=True)

            # ---- startup DMAs: weights on the Sync queue, x on the GpSimd
            # queue — two independent in-order queues (~350 GB/s each) so the
            # gate weights and x stream in parallel. fc0's wg/wu are split in
            # halves so the first matmuls can start after ~1MB. ----
            wg_sb = {}   # fc -> [128, 16*512] tile (all 16 d-tiles merged)
            wu_sb = {}
            wd_sb = {}   # ft -> [128, 2048] tile

            def load_w0(dram, name):
                wt = wgu_pool.tile([128, ND * 512], BF16, tag="w", name=name)
                for half in range(2):
                    nc.sync.dma_start(
                        wt[:, half * 8 * 512:(half + 1) * 8 * 512]
                        .rearrange("p (n f) -> p n f", n=8),
                        dram[half * 1024:(half + 1) * 1024, :512]
                        .rearrange("(n p) f -> p n f", p=128),
                    )
                return wt

            wg_sb[0] = load_w0(wg_d, "wgt")
            wu_sb[0] = load_w0(wu_d, "wut")

            xt4 = [x_pool.tile([128, 4 * C], BF16, tag=f"xt{q}", name=f"xt{q}")
                   for q in range(4)]
            for q in range(4):
                nc.gpsimd.dma_start(
                    xt4[q][:].rearrange("p (n c) -> p n c", n=4),
                    xb_d[q * 512:(q + 1) * 512, :]
                    .rearrange("(n p) c -> p n c", p=128),
                )

            def xsl(d, c0, cn):
                q, r = divmod(d, 4)
                return xt4[q][:, r * C + c0:r * C + c0 + cn]

            wvb = const_pool.tile([128, C], FP32, tag="wvb", name="wvb")
            ya = [ya_pool.tile([128, C], FP32, tag=f"ya{dt}", name=f"ya{dt}")
                  for dt in range(ND)]

            def load_w(dram, fc, name):
                wt = wgu_pool.tile([128, ND * 512], BF16, tag="w", name=name)
                nc.sync.dma_start(
                    wt[:].rearrange("p (n f) -> p n f", n=ND),
                    dram[:, fc * 512:(fc + 1) * 512]
                    .rearrange("(n p) f -> p n f", p=128),
                )
                return wt

            def emit_down(gprev, hprev, j):
                """Down-matmul (d-tiles 2j, 2j+1) for f-group gprev."""
                f0 = gprev * G
                for dt in (2 * j, 2 * j + 1):
                    for (c0, cn) in chunks:
                        py = psy_pool.tile([128, max(cn for _, cn in chunks)],
                                           FP32, tag="py", name="py")
                        for k in range(G):
                            nc.tensor.matmul(
                                py[:, :cn],
                                wd_sb[f0 + k][:, dt * 128:(dt + 1) * 128],
                                hprev[k][:, c0:c0 + cn],
                                start=(k == 0), stop=(k == G - 1),
                            )
                        yslc = ya[dt][:, c0:c0 + cn]
                        if gprev == 0:
                            nc.scalar.copy(yslc, py[:, :cn])
                        else:
                            nc.vector.tensor_tensor(yslc, yslc, py[:, :cn],
                                                    op=ALU.add)
                    if gprev == NG - 1:
                        nc.vector.tensor_mul(ya[dt][:], ya[dt][:], wvb[:])
                        nc.gpsimd.dma_start(
                            y_d[dt * 128:(dt + 1) * 128, :], ya[dt][:])

            def emit_gate(fc, fo):
                pg = [ps_pool.tile([128, cn], FP32, tag="ps", name="ps")
                      for (c0, cn) in chunks]
                for d in range(ND):
                    wsl = wg_sb[fc][:, d * 512 + fo * 128:
                                    d * 512 + (fo + 1) * 128]
                    for ci, (c0, cn) in enumerate(chunks):
                        nc.tensor.matmul(
                            pg[ci][:], wsl, xsl(d, c0, cn),
                            start=(d == 0), stop=(d == ND - 1),
                        )
                st = tmp_pool.tile([128, C], FP32, tag="st", name="st",
                                   bufs=4)
                for ci, (c0, cn) in enumerate(chunks):
                    nc.scalar.activation(st[:, c0:c0 + cn], pg[ci][:],
                                         ACTF.Silu)
                return st

            def emit_up(fc, fo, st, j):
                pu = [ps_pool.tile([128, cn], FP32, tag="ps", name="ps")
                      for (c0, cn) in chunks]
                for d in range(ND):
                    wsl = wu_sb[fc][:, d * 512 + fo * 128:
                                    d * 512 + (fo + 1) * 128]
                    for ci, (c0, cn) in enumerate(chunks):
                        nc.tensor.matmul(
                            pu[ci][:], wsl, xsl(d, c0, cn),
                            start=(d == 0), stop=(d == ND - 1),
                        )
                ht = h_pool.tile([128, C], BF16, tag=f"h{j}", name=f"h{j}")
                for ci, (c0, cn) in enumerate(chunks):
                    nc.vector.tensor_mul(ht[:, c0:c0 + cn],
                                         st[:, c0:c0 + cn], pu[ci][:])
                ft = (fc * 4 + fo)
                wdt = wd_pool.tile([128, D], BF16, tag=f"wd{j}", name="wdt")
                nc.sync.dma_start(wdt[:], wd_d[ft * 128:(ft + 1) * 128, :])
                wd_sb[ft] = wdt
                return ht

            hprev = None
            for g in range(NG):
                hcur = []
                for j in range(G):
                    ft = g * G + j
                    fc, fo = divmod(ft, 4)
                    if fc == 0:
                        # fc0: run all 4 gate f-tiles before the first up so
                        # the PE has work while the up-weights stream in.
                        if j == 0:
                            sts0 = [emit_gate(0, ff) for ff in range(4)]
                        ht = emit_up(0, fo, sts0[fo], j)
                    else:
                        if fo == 0:
                            wg_sb[fc] = load_w(wg_d, fc, "wgt")
                        st = emit_gate(fc, fo)
                        # up-weights DMA goes behind this f-tile's gate MMs
                        if fo == 0:
                            wu_sb[fc] = load_w(wu_d, fc, "wut")
                        ht = emit_up(fc, fo, st, j)
                    if g == NG - 1 and j == 0:
                        nc.gpsimd.dma_start(wvb[:], wvb_d[:])
                    hcur.append(ht)
                    if hprev is not None:
                        emit_down(g - 1, hprev, j)
                hprev = hcur
            for j in range(G):
                emit_down(NG - 1, hprev, j)

    nc.compile()
    return nc


_PROGRAM_CACHE = {}


def _get_program(C):
    if C not in _PROGRAM_CACHE:
        _PROGRAM_CACHE[C] = build_program(C)
    return _PROGRAM_CACHE[C]


def _route_host(x_TD, router_w):
    """Host router: top-2 ids + renormalized softmax weights per token."""
    logits = (x_TD @ router_w).astype(np.float64)  # [T, E]
    logits -= logits.max(axis=1, keepdims=True)
    probs = np.exp(logits)
    probs /= probs.sum(axis=1, keepdims=True)
    order = np.argsort(-probs, axis=1, kind="stable")
    top2 = order[:, :2]
    w12 = np.take_along_axis(probs, top2, axis=1)
    w12 /= w12.sum(axis=1, keepdims=True)
    return top2, w12.astype(np.float32)


def kernel_with_results(x_TD, router_w, w_gate, w_up, w_down):
    x_TD = np.ascontiguousarray(x_TD, np.float32)
    router_w = np.ascontiguousarray(router_w, np.float32)

    top2, w12 = _route_host(x_TD, router_w)
    idx_lists, wt_lists = [], []
    for e in range(E):
        sel = top2 == e  # [T, 2]
        any_sel = sel.any(axis=1)
        ix = np.where(any_sel)[0]
        idx_lists.append(ix)
        wt_lists.append(w12[any_sel][sel[ix]])
    max_cnt = max(len(ix) for ix in idx_lists)
    C = max(64, -(-max_cnt // 8) * 8)

    nc = _get_program(C)

    xT = np.ascontiguousarray(x_TD.T).astype(_BF16NP)  # [D, T] bf16
    in_maps = []
    for e in range(E):
        ix = idx_lists[e]
        xb = np.zeros((D, C), _BF16NP)
        xb[:, :len(ix)] = xT[:, ix]
        wvb = np.zeros((1, C), np.float32)
        wvb[0, :len(ix)] = wt_lists[e]
        in_maps.append({
            "xb": xb,
            "wvb": np.ascontiguousarray(np.broadcast_to(wvb, (128, C))),
            "wg": w_gate[e].astype(_BF16NP),
            "wu": w_up[e].astype(_BF16NP),
            "wd": w_down[e].astype(_BF16NP),
        })

    try:
        res = bass_utils.run_bass_kernel_spmd(
            nc, in_maps, core_ids=list(range(NCORES))
        )
    except ModuleNotFoundError:
        # Tracing requested via env but the axon NTFF hook module is absent
        # in this image — rerun without tracing.
        os.environ["BASS_NEVER_TRACE"] = "1"
        res = bass_utils.run_bass_kernel_spmd(
            nc, in_maps, core_ids=list(range(NCORES))
        )

    out = np.zeros((T, D), np.float32)
    for e in range(E):
        ix = idx_lists[e]
        y = res.results[e]["y"]  # [D, C]
        out[ix] += y[:, :len(ix)].T
    return out, res


def kernel(**inputs):
    out, _ = kernel_with_results(**inputs)
    return out
